# revision 1
# baseline (speedup 1.0000x reference)
"""MultiHeadAttention Trainium2 kernel (8-core SPMD, head/tensor parallel).

Problem (hardcoded shapes): stream (2048, 2, 1024) f32, mask (1, 2048, 2048),
w_qkv (1024, 3072), b_qkv (3072,), w_out (1024, 1024), b_out (1024,).
N=2048, B=2, HEADS=16, D_KQ=D_V=64, D_HEAD=192.

Sharding (per the b*heads head-parallel hint): core d handles batch b=d//4 and
the 4 heads [4*(d%4), 4*(d%4)+4): w_qkv columns and w_out rows are split per
head group, logits/weights are fully local per core, and the post-projection
all-reduce (sum over the 4 cores of each batch, + b_out) is done on the host
during unsharding.

Per-core compute, all in "transposed" orientation so no on-device transposes
are needed (the host pre-transposes stream and mask while sharding):

  qkT[f, n]   = (w_qkv_local.T @ x_b.T)[f, n] + b     (f = head-pair d dims)
  v[m, dv]    = (x_b @ w_v_local)[m, dv] + b_v        (bias via a K=1 matmul)
  logitsT     = per head: lT[m, n] = sum_d k[m,d] q[n,d]   (2 heads row-packed
                in the PE array: K=64 each at tile_position (0,0)/(64,0))
  wT[m, n]    = exp(lT) * exp(maskT)[m, n]            (unnormalized softmax;
                exp(mask) is precomputed on the host, applied as a bf16
                multiply at DVE 2x rate; no max-subtraction needed at these
                logit magnitudes)
  psv         = PV matmul with lhsT = [v | ones-block] so the output carries
                values^T rows plus 64x-replicated row-sums Z (the softmax
                denominator) in the complementary partition half, for free
  valT[hv, n] = psv_values * recip(Z)                 (recip on DVE; the recip
                block is DMA-moved across partitions; DVE is lane-locked)
  out_partial = valT^T @ w_out_local   -> DMA to HBM, host sums per batch

dtypes: float32r (full-rate fp32) for the projections, QK^T and the output
projection; bf16 only on the exp->mask->PV path where DVE 2x mode needs it.
PSUM is managed as 4 rotating 2-bank slots (A/B for logits+projections+output,
V0/V1 for the two PV accumulators of the active head pair).
"""

import numpy as np
import ml_dtypes

import concourse.tile as tile
from concourse import bacc, mybir
from concourse.bass_utils import run_bass_kernel_spmd

BF16 = ml_dtypes.bfloat16
dt = mybir.dt
AF = mybir.ActivationFunctionType

# Shapes (hardcoded per the problem spec)
N = 2048          # sequence length
B = 2             # batch
DSTR = 1024       # d_stream
HEADS = 16        # total heads
NH = 4            # heads per core
DKQ = 64
DV = 64
DHEAD = 2 * DKQ + DV
P = 128
KT = DSTR // P    # 8 contraction k-tiles for projections
MT = N // P       # 16 m-tiles
CH = 1024         # attention n-chunk width
NCH = N // CH     # 2 chunks
NB = 512          # matmul moving free dim
N_CORES = 8

f32, f32r, bf16 = dt.float32, dt.float32r, dt.bfloat16

_BUILT = {}


def _build_nc():
    """Build + compile the single-core SPMD Bass program (same on all cores)."""
    nc = bacc.Bacc("TRN2", target_bir_lowering=False, debug=False)

    xT = nc.dram_tensor("xT", [DSTR, N], f32r, kind="ExternalInput").ap()
    wqk = nc.dram_tensor("wqk", [DSTR, 4 * P], f32r, kind="ExternalInput").ap()
    wv = nc.dram_tensor("wv", [DSTR, NH * DV], f32r, kind="ExternalInput").ap()
    bqk = nc.dram_tensor("bqk", [P, 4], f32, kind="ExternalInput").ap()
    bv = nc.dram_tensor("bv", [1, NH * DV], f32r, kind="ExternalInput").ap()
    ones = nc.dram_tensor("ones", [1, P], f32r, kind="ExternalInput").ap()
    emT = nc.dram_tensor("emT", [N, N], bf16, kind="ExternalInput").ap()
    wout = nc.dram_tensor("wout", [NH * DV, DSTR], f32r, kind="ExternalInput").ap()
    out = nc.dram_tensor("out", [N, DSTR], f32, kind="ExternalOutput").ap()

    with tile.TileContext(nc) as tc:
        with (
            tc.tile_pool(name="consts", bufs=1) as consts,
            tc.tile_pool(name="xw", bufs=1) as xw_p,
            tc.tile_pool(name="qkT", bufs=1) as qkT_p,
            tc.tile_pool(name="v1", bufs=1) as v1_p,
            tc.tile_pool(name="valT", bufs=1) as valT_p,
            tc.tile_pool(name="mask", bufs=3) as mask_p,
            tc.tile_pool(name="wT", bufs=3) as wT_p,
            tc.tile_pool(name="z", bufs=1) as z_p,
            tc.tile_pool(name="ps", bufs=1, space="PSUM") as ps_p,
        ):
            # ---------- persistent SBUF ----------
            xT_sb = xw_p.tile([P, KT, N], f32r)
            wqk_sb = xw_p.tile([P, KT, 4 * P], f32r)
            wv_sb = xw_p.tile([P, KT, NH * DV], f32r)
            bqk_sb = consts.tile([P, 4], f32)
            nc.scalar.dma_start(out=bqk_sb, in_=bqk)
            for kt in range(KT):
                e1 = nc.sync if kt % 2 == 0 else nc.scalar
                e2 = nc.scalar if kt % 2 == 0 else nc.sync
                e1.dma_start(out=xT_sb[:, kt, :], in_=xT[kt * P:(kt + 1) * P, :])
                e2.dma_start(out=wqk_sb[:, kt, :], in_=wqk[kt * P:(kt + 1) * P, :])
                e2.dma_start(out=wv_sb[:, kt, :], in_=wv[kt * P:(kt + 1) * P, :])
            bv_sb = consts.tile([1, NH * DV], f32r)
            nc.sync.dma_start(out=bv_sb, in_=bv)
            ones1 = consts.tile([1, P], f32r)
            nc.sync.dma_start(out=ones1, in_=ones)
            wout_sb = consts.tile([P, 2, DSTR], f32r)
            nc.sync.dma_start(
                out=wout_sb, in_=wout.rearrange("(i p) d -> p i d", p=P))

            # qkT f-tiles: 0 = q pair0, 1 = q pair1, 2 = k pair0, 3 = k pair1
            # (within a tile: partitions 0:64 = even head's d, 64:128 = odd head's)
            qkT = qkT_p.tile([P, 4, N], f32r)
            # v1 lhsT slots per (mt, head): even-in-pair = [v | ones],
            # odd-in-pair = [ones | v] -> PV output carries values rows and
            # 64x-replicated Z rows in complementary partition halves.
            v1 = v1_p.tile([P, MT, NH, P], bf16)
            nc.vector.memset(v1[:, :, 0::2, 64:128], 1.0)
            nc.vector.memset(v1[:, :, 1::2, 0:64], 1.0)
            valT = [valT_p.tile([P, N], f32r, tag=f"valT{i}", name=f"valT{i}")
                    for i in range(2)]

            # ---------- projections ----------
            def proj_qk(ft):
                with nc.named_scope(f"proj_qk{ft}"):
                    for nb in range(N // NB):
                        ps = ps_p.tile([P, NB], f32, tag=["A", "B", "V0", "V1"][nb % 4])
                        for kt in range(KT):
                            nc.tensor.matmul(
                                ps,
                                lhsT=wqk_sb[:, kt, ft * P:(ft + 1) * P],
                                rhs=xT_sb[:, kt, nb * NB:(nb + 1) * NB],
                                start=(kt == 0), stop=(kt == KT - 1),
                            )
                        nc.scalar.activation(
                            out=qkT[:, ft, nb * NB:(nb + 1) * NB], in_=ps,
                            func=AF.Identity, bias=bqk_sb[:, ft:ft + 1],
                        )

            def proj_v():
                with nc.named_scope("proj_v"):
                    for mt in range(MT):
                        ps = ps_p.tile([P, NH * DV], f32, tag=["A", "B", "V0", "V1"][mt % 4])
                        for kt in range(KT):
                            nc.tensor.matmul(
                                ps,
                                lhsT=xT_sb[:, kt, mt * P:(mt + 1) * P],
                                rhs=wv_sb[:, kt, :],
                                start=(kt == 0), stop=False,
                            )
                        nc.tensor.matmul(
                            ps, lhsT=ones1, rhs=bv_sb,
                            start=False, stop=True,
                        )
                        psj = ps.rearrange("p (j d) -> p j d", d=DV)
                        nc.vector.tensor_copy(
                            out=v1[:, mt, 0::2, 0:DV], in_=psj[:, 0::2, :])
                        nc.vector.tensor_copy(
                            out=v1[:, mt, 1::2, 64:64 + DV], in_=psj[:, 1::2, :])

            # ---------- attention for one head pair over one n-chunk ----------
            def attn(p, c):
                with nc.named_scope(f"attn_p{p}_c{c}"):
                    psv = [
                        ps_p.tile([P, CH], f32, tag=f"V{oe}",
                                  name=f"psv{oe}_p{p}_c{c}")
                        for oe in (0, 1)
                    ]
                    for mt in range(MT):
                        em = mask_p.tile([P, CH], bf16)
                        nc.sync.dma_start(
                            out=em,
                            in_=emT[mt * P:(mt + 1) * P, c * CH:(c + 1) * CH],
                        )
                        for oe in (0, 1):
                            j = 2 * p + oe
                            base = oe * 64
                            psl = ps_p.tile([P, CH], f32, tag="AB"[oe])
                            for h2 in range(CH // NB):
                                nc.tensor.matmul(
                                    psl[:, h2 * NB:(h2 + 1) * NB],
                                    lhsT=qkT[base:base + 64, 2 + p,
                                                mt * P:(mt + 1) * P],
                                    rhs=qkT[base:base + 64, p,
                                               c * CH + h2 * NB:
                                               c * CH + (h2 + 1) * NB],
                                    start=True, stop=True,
                                )
                            wt = wT_p.tile([P, CH], bf16, bufs=4)
                            nc.scalar.activation(out=wt, in_=psl, func=AF.Exp)
                            nc.vector.tensor_mul(out=wt, in0=wt, in1=em)
                            for h2 in range(CH // NB):
                                nc.tensor.matmul(
                                    psv[oe][:, h2 * NB:(h2 + 1) * NB],
                                    lhsT=v1[:, mt, j, :],
                                    rhs=wt[:, h2 * NB:(h2 + 1) * NB],
                                    start=(mt == 0), stop=(mt == MT - 1),
                                )
                    # normalization: recip the replicated Z rows, DMA-move the
                    # reciprocal block to the values rows' partitions, multiply.
                    with nc.named_scope(f"norm_p{p}_c{c}"):
                        zr = z_p.tile([P, CH], f32, tag="zr")
                        zm = z_p.tile([P, CH], f32, tag="zm")
                        cs = slice(c * CH, (c + 1) * CH)
                        # even head: values rows 0:64, Z rows 64:128
                        nc.vector.reciprocal(out=zr[64:128, :], in_=psv[0][64:128, :])
                        nc.sync.dma_start(out=zm[0:64, :], in_=zr[64:128, :])
                        nc.vector.tensor_mul(
                            out=valT[p][0:64, cs], in0=psv[0][0:64, :],
                            in1=zm[0:64, :],
                        )
                        # odd head: Z rows 0:64, values rows 64:128
                        nc.vector.reciprocal(out=zr[0:64, :], in_=psv[1][0:64, :])
                        nc.sync.dma_start(out=zm[64:128, :], in_=zr[0:64, :])
                        nc.vector.tensor_mul(
                            out=valT[p][64:128, cs], in0=psv[1][64:128, :],
                            in1=zm[64:128, :],
                        )

            # ---------- output projection ----------
            def outproj():
                with nc.named_scope("outproj"):
                    for nt2 in range(MT // 2):
                        ob = wT_p.tile([P, 2, DSTR], f32, tag="outbuf")
                        for half in range(2):
                            nt = 2 * nt2 + half
                            for ds in range(DSTR // NB):
                                ps = ps_p.tile([P, NB], f32,
                                               tag="AB"[(2 * nt + ds) % 2])
                                for p in range(2):
                                    nc.tensor.matmul(
                                        ps,
                                        lhsT=valT[p][:, nt * P:(nt + 1) * P],
                                        rhs=wout_sb[:, p, ds * NB:(ds + 1) * NB],
                                        start=(p == 0), stop=(p == 1),
                                    )
                                obs = ob[:, half, ds * NB:(ds + 1) * NB]
                                if ds == 0:
                                    nc.scalar.copy(out=obs, in_=ps)
                                else:
                                    nc.vector.tensor_copy(out=obs, in_=ps)
                        eng = nc.sync
                        eng.dma_start(
                            out=out[nt2 * 2 * P:(nt2 + 1) * 2 * P, :].rearrange(
                                "(h p) d -> p h d", p=P),
                            in_=ob)

            proj_qk(0)
            proj_qk(2)
            proj_v()
            for c in range(NCH):
                attn(0, c)
            proj_qk(1)
            proj_qk(3)
            for c in range(NCH):
                attn(1, c)
            outproj()

    nc.compile()
    return nc


def get_nc():
    if "nc" not in _BUILT:
        _BUILT["nc"] = _build_nc()
    return _BUILT["nc"]


def _shard_inputs(stream, mask, w_qkv, b_qkv, w_out):
    """Build the 8 per-core input maps (host-side layout transforms)."""
    stream = np.asarray(stream, np.float32)
    mask = np.asarray(mask, np.float32)
    w_qkv = np.asarray(w_qkv, np.float32)
    b_qkv = np.asarray(b_qkv, np.float32)
    w_out = np.asarray(w_out, np.float32)

    emT = np.exp(mask[0].T).astype(BF16)  # (N, N) exp of transposed mask
    xT = [np.ascontiguousarray(stream[:, b, :].T) for b in range(B)]

    in_maps = []
    for d in range(N_CORES):
        b = d // 4
        heads = [(d % 4) * 4 + j for j in range(NH)]
        qc = [w_qkv[:, h * DHEAD:h * DHEAD + DKQ] for h in heads]
        kc = [w_qkv[:, h * DHEAD + DKQ:h * DHEAD + 2 * DKQ] for h in heads]
        vc = [w_qkv[:, h * DHEAD + 2 * DKQ:(h + 1) * DHEAD] for h in heads]
        wqk = np.ascontiguousarray(np.concatenate(
            [qc[0], qc[1], qc[2], qc[3], kc[0], kc[1], kc[2], kc[3]], axis=1))
        wv = np.ascontiguousarray(np.concatenate(vc, axis=1))
        bq = [b_qkv[h * DHEAD:h * DHEAD + DKQ] for h in heads]
        bk = [b_qkv[h * DHEAD + DKQ:h * DHEAD + 2 * DKQ] for h in heads]
        bvv = [b_qkv[h * DHEAD + 2 * DKQ:(h + 1) * DHEAD] for h in heads]
        bqk_arr = np.stack(
            [np.concatenate([bq[0], bq[1]]), np.concatenate([bq[2], bq[3]]),
             np.concatenate([bk[0], bk[1]]), np.concatenate([bk[2], bk[3]])],
            axis=1).astype(np.float32)
        bv_arr = np.ascontiguousarray(np.concatenate(bvv)[None, :])
        woutd = np.ascontiguousarray(
            np.concatenate([w_out[h * DV:(h + 1) * DV, :] for h in heads], axis=0))
        in_maps.append({
            "xT": xT[b], "wqk": wqk, "wv": wv, "bqk": bqk_arr, "bv": bv_arr,
            "ones": np.ones((1, P), np.float32), "emT": emT, "wout": woutd,
        })
    return in_maps


def kernel(stream, mask, w_qkv, b_qkv, w_out, b_out):
    nc = get_nc()
    in_maps = _shard_inputs(stream, mask, w_qkv, b_qkv, w_out)
    res = run_bass_kernel_spmd(nc, in_maps, core_ids=list(range(N_CORES)))
    b_out = np.asarray(b_out, np.float32)
    out = np.empty((N, B, DSTR), np.float32)
    for b in range(B):
        acc = res.results[4 * b]["out"].copy()
        for i in range(1, 4):
            acc += res.results[4 * b + i]["out"]
        out[:, b, :] = acc + b_out
    return out



# revision 40
# speedup vs baseline: 1.1494x; 1.1494x over previous
"""MultiHeadAttention Trainium2 kernel (8-core SPMD, head/tensor parallel).

Problem (hardcoded shapes): stream (2048, 2, 1024) f32, mask (1, 2048, 2048),
w_qkv (1024, 3072), b_qkv (3072,), w_out (1024, 1024), b_out (1024,).
N=2048, B=2, HEADS=16, D_KQ=D_V=64, D_HEAD=192.

Sharding (per the b*heads head-parallel hint): core d handles batch b=d//4 and
the 4 heads [4*(d%4), 4*(d%4)+4): w_qkv columns and w_out rows are split per
head group, logits/weights are fully local per core, and the post-projection
all-reduce (sum over the 4 cores of each batch plus the two per-pair partial
outputs, + b_out) is done on the host during unsharding.

Schedule (engine-balanced, derived from the TimelineSim cost model):
  - ACT only runs the 128 softmax exps ([128,1024] each, ~133us total); all
    bias-adds and PSUM->SBUF copies live on DVE/Pool so ACT is never stalled.
  - Projection inputs (xT/w_qkv/w_v) are bf16: halves the startup DMA and
    keeps PE continuously busy (p-state ramp).
  - QKV projection for the first head pair runs kt-major across four
    [128,1024] PSUM tiles so each arriving xT k-slice is consumed at once.
  - Attention runs as 8 windows of (pair, chunk, head): single PV accumulator
    per window leaves PSUM tag F free for interleaved filler matmuls (second
    pair's q/k projection, per-pair output projection) that soak up the
    PE idle gap while ACT streams exps.
  - exp(mask^T) is SBUF-resident (loaded once, bf16, applied as a DVE 2x/4x
    multiply); softmax denominators come free from ones-columns in the PV
    lhsT; reciprocal on DVE with a DMA partition-move.
  - Output projection is per head-pair (no cross-pair PSUM accumulation) so
    each pair's contribution streams out as soon as that pair's values are
    normalized; the host sums the two partial outputs per core.
"""

import numpy as np
import ml_dtypes

import concourse.tile as tile
from concourse import bacc, mybir
from concourse.bass_utils import run_bass_kernel_spmd

BF16 = ml_dtypes.bfloat16
dt = mybir.dt
AF = mybir.ActivationFunctionType

# Shapes (hardcoded per the problem spec)
N = 2048          # sequence length
B = 2             # batch
DSTR = 1024       # d_stream
HEADS = 16        # total heads
NH = 4            # heads per core
DKQ = 64
DV = 64
DHEAD = 2 * DKQ + DV
P = 128
KT = DSTR // P    # 8 contraction k-tiles for projections
MT = N // P       # 16 m-tiles
CH = 1024         # attention n-chunk width
NCH = N // CH     # 2 chunks
NB = 512          # matmul moving free dim
N_CORES = 8

f32, f32r, bf16 = dt.float32, dt.float32r, dt.bfloat16

_BUILT = {}


def _build_nc():
    """Build + compile the single-core SPMD Bass program (same on all cores)."""
    nc = bacc.Bacc("TRN2", target_bir_lowering=False, debug=False)

    xT = nc.dram_tensor("xT", [DSTR, N], bf16, kind="ExternalInput").ap()
    wqk = nc.dram_tensor("wqk", [DSTR, 4 * P], bf16, kind="ExternalInput").ap()
    wv = nc.dram_tensor("wv", [DSTR, NH * DV], bf16, kind="ExternalInput").ap()
    bqk = nc.dram_tensor("bqk", [P, 4], f32, kind="ExternalInput").ap()
    bv = nc.dram_tensor("bv", [1, NH * DV], bf16, kind="ExternalInput").ap()
    ones = nc.dram_tensor("ones", [1, P], bf16, kind="ExternalInput").ap()
    emT = nc.dram_tensor("emT", [N, N], bf16, kind="ExternalInput").ap()
    wout = nc.dram_tensor("wout", [NH * DV, DSTR], bf16, kind="ExternalInput").ap()
    outp = [
        nc.dram_tensor(f"out{i}", [N, DSTR], bf16, kind="ExternalOutput").ap()
        for i in range(2)
    ]

    with tile.TileContext(nc) as tc:
        with (
            tc.tile_pool(name="consts", bufs=1) as consts,
            tc.tile_pool(name="xw", bufs=1) as xw_p,
            tc.tile_pool(name="qkT", bufs=1) as qkT_p,
            tc.tile_pool(name="v1", bufs=1) as v1_p,
            tc.tile_pool(name="valT", bufs=1) as valT_p,
            tc.tile_pool(name="em", bufs=1) as em_p,
            tc.tile_pool(name="wT", bufs=4) as wT_p,
            tc.tile_pool(name="z", bufs=2) as z_p,
            tc.tile_pool(name="stage", bufs=8) as stage_p,
            tc.tile_pool(name="ps", bufs=1, space="PSUM") as ps_p,
        ):
            # ---------- persistent SBUF ----------
            xT_sb = xw_p.tile([P, KT, N], bf16)
            wqk_sb = xw_p.tile([P, KT, 4 * P], bf16)
            wv_sb = xw_p.tile([P, KT, NH * DV], bf16)
            for kt in range(KT):
                nc.sync.dma_start(out=wqk_sb[:, kt, :], in_=wqk[kt * P:(kt + 1) * P, :])
                nc.sync.dma_start(out=xT_sb[:, kt, :], in_=xT[kt * P:(kt + 1) * P, :])
                nc.sync.dma_start(out=wv_sb[:, kt, :], in_=wv[kt * P:(kt + 1) * P, :])
            bqk_sb = consts.tile([P, 4], f32)
            nc.sync.dma_start(out=bqk_sb, in_=bqk)
            # pre-warm the ACT exp table off the critical path
            warm = consts.tile([P, 1], f32)
            nc.scalar.activation(out=warm, in_=bqk_sb[:, 0:1], func=AF.Exp)
            ones1 = consts.tile([1, P], bf16)
            nc.sync.dma_start(out=ones1, in_=ones)
            bv_sb = consts.tile([1, NH * DV], bf16)
            nc.sync.dma_start(out=bv_sb, in_=bv)
            wout_sb = consts.tile([P, 2, DSTR], bf16)
            nc.sync.dma_start(
                out=wout_sb, in_=wout.rearrange("(i p) d -> p i d", p=P))

            # exp(mask^T), SBUF-resident for the whole kernel
            em_sb = em_p.tile([P, NCH, MT, CH], bf16)
            for c in range(NCH):
                for q in range(4):
                    nc.sync.dma_start(
                        out=em_sb[:, c, 4 * q:4 * (q + 1), :],
                        in_=emT[4 * q * P:4 * (q + 1) * P,
                                c * CH:(c + 1) * CH].rearrange(
                                    "(t p) n -> p t n", p=P),
                    )

            # qkT f-tiles: 0 = q pair0, 1 = q pair1, 2 = k pair0, 3 = k pair1
            # (within a tile: partitions 0:64 = even head's d, 64:128 = odd's)
            qkT = qkT_p.tile([P, 4, N], bf16)
            # v1 lhsT slots per (mt, head): even-in-pair = [v | ones],
            # odd-in-pair = [ones | v] -> PV output carries values rows and
            # 64x-replicated Z rows in complementary partition halves.
            v1 = v1_p.tile([P, MT, NH, P], bf16)
            nc.gpsimd.memset(v1[:, :, 0::2, 64:128], 1.0)
            nc.gpsimd.memset(v1[:, :, 1::2, 0:64], 1.0)
            valT = [valT_p.tile([P, N], bf16, tag=f"valT{i}", name=f"valT{i}")
                    for i in range(2)]

            TAGS = ["A", "B", "V", "F"]

            # ---------- phase A: kt-major projection of q/k for pair 0 ----------
            with nc.named_scope("proj_qk02"):
                pt = {}
                for i, (ft, half) in enumerate([(0, 0), (0, 1), (2, 0), (2, 1)]):
                    pt[(ft, half)] = ps_p.tile(
                        [P, CH], f32, tag=TAGS[i], name=f"pqk_{ft}_{half}")
                for kt in range(KT):
                    for (ft, half), t in pt.items():
                        for h2 in range(2):
                            nb = half * 2 + h2
                            nc.tensor.matmul(
                                t[:, h2 * NB:(h2 + 1) * NB],
                                lhsT=wqk_sb[:, kt, ft * P:(ft + 1) * P],
                                rhs=xT_sb[:, kt, nb * NB:(nb + 1) * NB],
                                start=(kt == 0), stop=(kt == KT - 1),
                            )
                for (ft, half), t in pt.items():
                    # ACT is idle during phase A: bias+copy in one activation
                    nc.scalar.activation(
                        out=qkT[:, ft, half * CH:(half + 1) * CH], in_=t,
                        func=AF.Identity, bias=bqk_sb[:, ft:ft + 1])

            # ---------- v projection: one m-tile (phase A seeds, rest are
            # pre-PV fillers inside window 0) ----------
            def pv_unit(mt, tag="F"):
                with nc.named_scope(f"proj_v_{mt}"):
                    t = ps_p.tile([P, NH * DV], f32, tag=tag)
                    for kt in range(KT):
                        nc.tensor.matmul(
                            t,
                            lhsT=xT_sb[:, kt, mt * P:(mt + 1) * P],
                            rhs=wv_sb[:, kt, :],
                            start=(kt == 0), stop=False,
                        )
                    nc.tensor.matmul(t, lhsT=ones1, rhs=bv_sb,
                                     start=False, stop=True)
                    psj = t.rearrange("p (j d) -> p j d", d=DV)
                    nc.vector.tensor_copy(
                        out=v1[:, mt, 0::2, 0:DV], in_=psj[:, 0::2, :])
                    nc.vector.tensor_copy(
                        out=v1[:, mt, 1::2, 64:64 + DV], in_=psj[:, 1::2, :])

            with nc.named_scope("proj_v_seed"):
                for mt in range(4):
                    pv_unit(mt, tag=TAGS[mt % 4])

            # ---------- filler units (run inside attention windows) ----------
            def qk_unit(ft, nb, tag="F"):
                """Project one 512-wide n-block of q or k for head pair 1."""
                with nc.named_scope(f"fqk_{ft}_{nb}"):
                    t = ps_p.tile([P, NB], f32, tag=tag)
                    for kt in range(KT):
                        nc.tensor.matmul(
                            t,
                            lhsT=wqk_sb[:, kt, ft * P:(ft + 1) * P],
                            rhs=xT_sb[:, kt, nb * NB:(nb + 1) * NB],
                            start=(kt == 0), stop=(kt == KT - 1),
                        )
                    nc.vector.tensor_scalar_add(
                        out=qkT[:, ft, nb * NB:(nb + 1) * NB],
                        in0=t, scalar1=bqk_sb[:, ft:ft + 1])

            _op_i = [0]

            def op_unit(p, nt, tag="F", act_copy=None):
                """Output-project one 128-row n-block of head pair p."""
                with nc.named_scope(f"fop_{p}_{nt}"):
                    t = ps_p.tile([P, CH], f32, tag=tag)
                    for ds in range(2):
                        nc.tensor.matmul(
                            t[:, ds * NB:(ds + 1) * NB],
                            lhsT=valT[p][:, nt * P:(nt + 1) * P],
                            rhs=wout_sb[:, p, ds * NB:(ds + 1) * NB],
                            start=True, stop=True,
                        )
                    ob = stage_p.tile([P, CH], bf16)
                    # every 4th staging copy rides ACT's per-window slack
                    if act_copy is None:
                        act_copy = _op_i[0] % 4 == 3
                    if act_copy:
                        nc.scalar.copy(out=ob, in_=t)
                    else:
                        nc.vector.tensor_copy(out=ob, in_=t)
                    _op_i[0] += 1
                    nc.sync.dma_start(out=outp[p][nt * P:(nt + 1) * P, :], in_=ob)

            # ---------- attention window: one (pair, chunk, head) ----------
            # Windows alternate their PV accumulator between PSUM tags V and F
            # (double-buffered); fillers use the off-duty tag. Each window's
            # normalization is emitted inside the NEXT window (as a pre-PV
            # filler at iteration 2) so the window boundary never injects
            # DVE latency into the exp->mul->PV chain.
            _widx = [0]

            def attn(p, c, oe, fillers=(), pre=None, last=False):
                j = 2 * p + oe
                base = oe * 64
                vtag = "VF"[_widx[0] % 2]
                ftag = "VF"[1 - _widx[0] % 2]
                _widx[0] += 1
                fill = {}
                if fillers:
                    step = max(1, 10 // len(fillers))
                    for i, f in enumerate(fillers):
                        fill.setdefault(min(13, 4 + i * step), []).append(
                            lambda f=f: f(ftag))
                pre = pre or {}
                with nc.named_scope(f"attn_{p}_{c}_{oe}"):
                    psv = ps_p.tile([P, CH], f32, tag=vtag, name=f"psv_{p}_{c}_{oe}")
                    for mt in range(MT):
                        psl = ps_p.tile([P, CH], f32, tag="AB"[mt % 2])
                        for h2 in range(2):
                            nc.tensor.matmul(
                                psl[:, h2 * NB:(h2 + 1) * NB],
                                lhsT=qkT[base:base + 64, 2 + p,
                                         mt * P:(mt + 1) * P],
                                rhs=qkT[base:base + 64, p,
                                        c * CH + h2 * NB:c * CH + (h2 + 1) * NB],
                                start=True, stop=True,
                            )
                        wt = wT_p.tile([P, CH], bf16)
                        nc.scalar.activation(out=wt, in_=psl, func=AF.Exp)
                        nc.vector.tensor_mul(out=wt, in0=wt, in1=em_sb[:, c, mt, :])
                        for f in pre.get(mt, ()):
                            f(ftag)
                        for h2 in range(2):
                            nc.tensor.matmul(
                                psv[:, h2 * NB:(h2 + 1) * NB],
                                lhsT=v1[:, mt, j, :],
                                rhs=wt[:, h2 * NB:(h2 + 1) * NB],
                                start=(mt == 0), stop=(mt == MT - 1),
                            )
                        for f in fill.get(mt, ()):
                            f()

                def finish(_tag=None):
                    # normalization: recip the replicated Z rows off psv,
                    # DMA-move across partitions (DVE is lane-locked), copy
                    # the value half out of PSUM, multiply on Pool.
                    with nc.named_scope(f"norm_{p}_{c}_{oe}"):
                        vb, zb = (0, 64) if oe == 0 else (64, 0)
                        cs = slice(c * CH, (c + 1) * CH)
                        zr = z_p.tile([P, CH], f32, tag="zr")
                        zm = z_p.tile([P, CH], f32, tag="zm")
                        nc.vector.reciprocal(
                            out=zr[zb:zb + 64, :], in_=psv[zb:zb + 64, :])
                        # issue via SWDGE (gpsimd): never queues behind the
                        # output stores on the sync queue
                        nc.gpsimd.dma_start(
                            out=zm[vb:vb + 64, :], in_=zr[zb:zb + 64, :])
                        if last:
                            # final window: multiply straight out of PSUM,
                            # split across DVE halves for minimum latency
                            nc.vector.tensor_mul(
                                out=valT[p][vb:vb + 64, cs][:, 0:NB],
                                in0=psv[vb:vb + 64, 0:NB],
                                in1=zm[vb:vb + 64, 0:NB])
                            nc.vector.tensor_mul(
                                out=valT[p][vb:vb + 64, cs][:, NB:CH],
                                in0=psv[vb:vb + 64, NB:CH],
                                in1=zm[vb:vb + 64, NB:CH])
                        else:
                            vc = z_p.tile([P, CH], f32, tag="pc")
                            nc.vector.tensor_copy(
                                out=vc[vb:vb + 64, :], in_=psv[vb:vb + 64, :])
                            nc.gpsimd.tensor_mul(
                                out=valT[p][vb:vb + 64, cs],
                                in0=vc[vb:vb + 64, :], in1=zm[vb:vb + 64, :])

                if last:
                    finish()
                    return None
                return finish

            # ---------- window schedule with interleaved fillers ----------
            # window 0 carries the rest of the v projection as pre-PV fillers
            # (v1[mt] must exist before its own PV consumes it; 4-ahead lead)
            def qk_f(ft, nb):
                return lambda tag: qk_unit(ft, nb, tag)

            def op_f(p, nt):
                return lambda tag: op_unit(p, nt, tag)

            n0 = attn(0, 0, 0,
                      pre={mt: [lambda tag, mt=mt: pv_unit(mt + 4, tag)]
                           for mt in range(MT - 4)})
            n1 = attn(0, 0, 1, [qk_f(1, nb) for nb in range(4)],
                      pre={2: [n0]})
            n2 = attn(0, 1, 0, [qk_f(3, 0), qk_f(3, 1),
                                op_f(0, 0), op_f(0, 1)],
                      pre={2: [n1]})
            n3 = attn(0, 1, 1, [qk_f(3, 2), qk_f(3, 3),
                                op_f(0, 2), op_f(0, 3)],
                      pre={2: [n2]})
            n4 = attn(1, 0, 0, [op_f(0, nt) for nt in (4, 5, 6, 7, 8, 9)],
                      pre={2: [n3]})
            n5 = attn(1, 0, 1, [op_f(0, nt) for nt in (10, 11, 12, 13, 14, 15)],
                      pre={2: [n4]})
            n6 = attn(1, 1, 0, [op_f(1, nt) for nt in range(6)],
                      pre={2: [n5]})
            attn(1, 1, 1, [op_f(1, 6), op_f(1, 7)],
                 pre={2: [n6]}, last=True)
            # tail: last chunk of pair 1, rotating through freed PSUM tags;
            # copies alternate DVE/ACT (both idle by now)
            with nc.named_scope("op_tail"):
                for i, nt in enumerate(range(8, 16)):
                    op_unit(1, nt, tag=["A", "B", "V", "F"][i % 4],
                            act_copy=(i % 2 == 1))

    nc.compile()
    return nc


def get_nc():
    if "nc" not in _BUILT:
        _BUILT["nc"] = _build_nc()
    return _BUILT["nc"]


def _shard_inputs(stream, mask, w_qkv, b_qkv, w_out):
    """Build the 8 per-core input maps (host-side layout transforms)."""
    stream = np.asarray(stream, np.float32)
    mask = np.asarray(mask, np.float32)
    w_qkv = np.asarray(w_qkv, np.float32)
    b_qkv = np.asarray(b_qkv, np.float32)
    w_out = np.asarray(w_out, np.float32)

    emT = np.exp(mask[0].T).astype(BF16)  # (N, N) exp of transposed mask
    xT = [np.ascontiguousarray(stream[:, b, :].T).astype(BF16) for b in range(B)]

    in_maps = []
    for d in range(N_CORES):
        b = d // 4
        heads = [(d % 4) * 4 + j for j in range(NH)]
        qc = [w_qkv[:, h * DHEAD:h * DHEAD + DKQ] for h in heads]
        kc = [w_qkv[:, h * DHEAD + DKQ:h * DHEAD + 2 * DKQ] for h in heads]
        vc = [w_qkv[:, h * DHEAD + 2 * DKQ:(h + 1) * DHEAD] for h in heads]
        wqk = np.ascontiguousarray(np.concatenate(
            [qc[0], qc[1], qc[2], qc[3], kc[0], kc[1], kc[2], kc[3]],
            axis=1)).astype(BF16)
        wv = np.ascontiguousarray(np.concatenate(vc, axis=1)).astype(BF16)
        bq = [b_qkv[h * DHEAD:h * DHEAD + DKQ] for h in heads]
        bk = [b_qkv[h * DHEAD + DKQ:h * DHEAD + 2 * DKQ] for h in heads]
        bvv = [b_qkv[h * DHEAD + 2 * DKQ:(h + 1) * DHEAD] for h in heads]
        bqk_arr = np.stack(
            [np.concatenate([bq[0], bq[1]]), np.concatenate([bq[2], bq[3]]),
             np.concatenate([bk[0], bk[1]]), np.concatenate([bk[2], bk[3]])],
            axis=1).astype(np.float32)
        bv_arr = np.ascontiguousarray(np.concatenate(bvv)[None, :]).astype(BF16)
        woutd = np.ascontiguousarray(np.concatenate(
            [w_out[h * DV:(h + 1) * DV, :] for h in heads], axis=0)).astype(BF16)
        in_maps.append({
            "xT": xT[b], "wqk": wqk, "wv": wv, "bqk": bqk_arr, "bv": bv_arr,
            "ones": np.ones((1, P), BF16), "emT": emT, "wout": woutd,
        })
    return in_maps


def kernel(stream, mask, w_qkv, b_qkv, w_out, b_out):
    nc = get_nc()
    in_maps = _shard_inputs(stream, mask, w_qkv, b_qkv, w_out)
    res = run_bass_kernel_spmd(nc, in_maps, core_ids=list(range(N_CORES)))
    b_out = np.asarray(b_out, np.float32)
    out = np.empty((N, B, DSTR), np.float32)
    for b in range(B):
        acc = res.results[4 * b]["out0"].astype(np.float32)
        acc += res.results[4 * b]["out1"].astype(np.float32)
        for i in range(1, 4):
            acc += res.results[4 * b + i]["out0"].astype(np.float32)
            acc += res.results[4 * b + i]["out1"].astype(np.float32)
        out[:, b, :] = acc + b_out
    return out


# revision 56
# speedup vs baseline: 1.1752x; 1.0225x over previous
"""MultiHeadAttention Trainium2 kernel (8-core SPMD, head/tensor parallel).

Problem (hardcoded shapes): stream (2048, 2, 1024) f32, mask (1, 2048, 2048),
w_qkv (1024, 3072), b_qkv (3072,), w_out (1024, 1024), b_out (1024,).
N=2048, B=2, HEADS=16, D_KQ=D_V=64, D_HEAD=192.

Sharding (per the b*heads head-parallel hint): core d handles batch b=d//4 and
the 4 heads [4*(d%4), 4*(d%4)+4): w_qkv columns and w_out rows are split per
head group, logits/weights are fully local per core, and the post-projection
all-reduce (sum over the 4 cores of each batch plus the two per-pair partial
outputs, + b_out) is done on the host during unsharding.

Schedule (engine-balanced, derived from the TimelineSim cost model):
  - ACT only runs the 128 softmax exps ([128,1024] each, ~133us total); all
    bias-adds and PSUM->SBUF copies live on DVE/Pool so ACT is never stalled.
  - Projection inputs (xT/w_qkv/w_v) are bf16: halves the startup DMA and
    keeps PE continuously busy (p-state ramp).
  - QKV projection for the first head pair runs kt-major across four
    [128,1024] PSUM tiles so each arriving xT k-slice is consumed at once.
  - Attention runs as 8 windows of (pair, chunk, head): single PV accumulator
    per window leaves PSUM tag F free for interleaved filler matmuls (second
    pair's q/k projection, per-pair output projection) that soak up the
    PE idle gap while ACT streams exps.
  - exp(mask^T) is SBUF-resident (loaded once, bf16, applied as a DVE 2x/4x
    multiply); softmax denominators come free from ones-columns in the PV
    lhsT; reciprocal on DVE with a DMA partition-move.
  - Output projection is per head-pair (no cross-pair PSUM accumulation) so
    each pair's contribution streams out as soon as that pair's values are
    normalized; the host sums the two partial outputs per core.
"""

import numpy as np
import ml_dtypes

import concourse.tile as tile
from concourse import bacc, mybir
from concourse.bass_utils import run_bass_kernel_spmd

BF16 = ml_dtypes.bfloat16
dt = mybir.dt
AF = mybir.ActivationFunctionType

# Shapes (hardcoded per the problem spec)
N = 2048          # sequence length
B = 2             # batch
DSTR = 1024       # d_stream
HEADS = 16        # total heads
NH = 4            # heads per core
DKQ = 64
DV = 64
DHEAD = 2 * DKQ + DV
P = 128
KT = DSTR // P    # 8 contraction k-tiles for projections
MT = N // P       # 16 m-tiles
CH = 1024         # attention n-chunk width
NCH = N // CH     # 2 chunks
NB = 512          # matmul moving free dim
N_CORES = 8

f32, f32r, bf16 = dt.float32, dt.float32r, dt.bfloat16

_BUILT = {}


def _build_nc():
    """Build + compile the single-core SPMD Bass program (same on all cores)."""
    nc = bacc.Bacc("TRN2", target_bir_lowering=False, debug=False)

    xT = nc.dram_tensor("xT", [DSTR, N], bf16, kind="ExternalInput").ap()
    wqk = nc.dram_tensor("wqk", [DSTR, 4 * P], bf16, kind="ExternalInput").ap()
    wv = nc.dram_tensor("wv", [DSTR, NH * DV], bf16, kind="ExternalInput").ap()
    bqk = nc.dram_tensor("bqk", [P, 4], f32, kind="ExternalInput").ap()
    bv = nc.dram_tensor("bv", [1, NH * DV], bf16, kind="ExternalInput").ap()
    ones = nc.dram_tensor("ones", [1, P], bf16, kind="ExternalInput").ap()
    emT = nc.dram_tensor("emT", [N, N], bf16, kind="ExternalInput").ap()
    wout = nc.dram_tensor("wout", [NH * DV, DSTR], bf16, kind="ExternalInput").ap()
    ident = nc.dram_tensor("ident", [P, 64], f32, kind="ExternalInput").ap()
    outp = [
        nc.dram_tensor(f"out{i}", [N, DSTR], bf16, kind="ExternalOutput").ap()
        for i in range(2)
    ]

    with tile.TileContext(nc) as tc:
        with (
            tc.tile_pool(name="consts", bufs=1) as consts,
            tc.tile_pool(name="xw", bufs=1) as xw_p,
            tc.tile_pool(name="qkT", bufs=1) as qkT_p,
            tc.tile_pool(name="v1", bufs=1) as v1_p,
            tc.tile_pool(name="valT", bufs=1) as valT_p,
            tc.tile_pool(name="em", bufs=1) as em_p,
            tc.tile_pool(name="wT", bufs=4) as wT_p,
            tc.tile_pool(name="z", bufs=2) as z_p,
            tc.tile_pool(name="stage", bufs=8) as stage_p,
            tc.tile_pool(name="ps", bufs=1, space="PSUM") as ps_p,
        ):
            # ---------- persistent SBUF ----------
            xT_sb = xw_p.tile([P, KT, N], bf16)
            wqk_sb = xw_p.tile([P, KT, 4 * P], bf16)
            wv_sb = xw_p.tile([P, KT, NH * DV], bf16)
            for kt in range(KT):
                nc.sync.dma_start(out=wqk_sb[:, kt, :], in_=wqk[kt * P:(kt + 1) * P, :])
                nc.sync.dma_start(out=xT_sb[:, kt, :], in_=xT[kt * P:(kt + 1) * P, :])
                nc.sync.dma_start(out=wv_sb[:, kt, :], in_=wv[kt * P:(kt + 1) * P, :])
            bqk_sb = consts.tile([P, 4], f32)
            nc.sync.dma_start(out=bqk_sb, in_=bqk)
            # pre-warm the ACT exp table off the critical path
            warm = consts.tile([P, 1], f32)
            nc.scalar.activation(out=warm, in_=bqk_sb[:, 0:1], func=AF.Exp)
            ones1 = consts.tile([1, P], bf16)
            nc.sync.dma_start(out=ones1, in_=ones)
            bv_sb = consts.tile([1, NH * DV], bf16)
            nc.sync.dma_start(out=bv_sb, in_=bv)
            wout_sb = consts.tile([P, 2, DSTR], bf16)
            nc.sync.dma_start(
                out=wout_sb, in_=wout.rearrange("(i p) d -> p i d", p=P))
            # 64x64 identity in both partition halves: PE-based partition
            # move of the final window's reciprocal block
            ident_sb = consts.tile([P, 64], f32)
            nc.sync.dma_start(out=ident_sb, in_=ident)

            # exp(mask^T), SBUF-resident for the whole kernel
            em_sb = em_p.tile([P, NCH, MT, CH], bf16)
            for c in range(NCH):
                for q in range(4):
                    nc.sync.dma_start(
                        out=em_sb[:, c, 4 * q:4 * (q + 1), :],
                        in_=emT[4 * q * P:4 * (q + 1) * P,
                                c * CH:(c + 1) * CH].rearrange(
                                    "(t p) n -> p t n", p=P),
                    )

            # qkT f-tiles: 0 = q pair0, 1 = q pair1, 2 = k pair0, 3 = k pair1
            # (within a tile: partitions 0:64 = even head's d, 64:128 = odd's)
            qkT = qkT_p.tile([P, 4, N], bf16)
            # v1 lhsT slots per (mt, head): even-in-pair = [v | ones],
            # odd-in-pair = [ones | v] -> PV output carries values rows and
            # 64x-replicated Z rows in complementary partition halves.
            v1 = v1_p.tile([P, MT, NH, P], bf16)
            nc.gpsimd.memset(v1[:, :, 0::2, 64:128], 1.0)
            nc.gpsimd.memset(v1[:, :, 1::2, 0:64], 1.0)
            valT = [valT_p.tile([P, N], bf16, tag=f"valT{i}", name=f"valT{i}")
                    for i in range(2)]

            TAGS = ["A", "B", "V", "F"]

            # ---------- phase A: kt-major projection of q/k for pair 0 ----------
            with nc.named_scope("proj_qk02"):
                pt = {}
                for i, (ft, half) in enumerate([(0, 0), (0, 1), (2, 0), (2, 1)]):
                    pt[(ft, half)] = ps_p.tile(
                        [P, CH], f32, tag=TAGS[i], name=f"pqk_{ft}_{half}")
                for kt in range(KT):
                    # nb01 (first xT half) before nb23, matching the split DMA
                    for half in range(2):
                        for ft in (0, 2):
                            t = pt[(ft, half)]
                            for h2 in range(2):
                                nb = half * 2 + h2
                                nc.tensor.matmul(
                                    t[:, h2 * NB:(h2 + 1) * NB],
                                    lhsT=wqk_sb[:, kt, ft * P:(ft + 1) * P],
                                    rhs=xT_sb[:, kt, nb * NB:(nb + 1) * NB],
                                    start=(kt == 0), stop=(kt == KT - 1),
                                )
                for (ft, half), t in pt.items():
                    # ACT is idle during phase A: bias+copy in one activation
                    nc.scalar.activation(
                        out=qkT[:, ft, half * CH:(half + 1) * CH], in_=t,
                        func=AF.Identity, bias=bqk_sb[:, ft:ft + 1])

            # ---------- v projection: one m-tile (phase A seeds, rest are
            # pre-PV fillers inside window 0) ----------
            def pv_unit(mt, tag="F"):
                with nc.named_scope(f"proj_v_{mt}"):
                    t = ps_p.tile([P, NH * DV], f32, tag=tag)
                    for kt in range(KT):
                        nc.tensor.matmul(
                            t,
                            lhsT=xT_sb[:, kt, mt * P:(mt + 1) * P],
                            rhs=wv_sb[:, kt, :],
                            start=(kt == 0), stop=False,
                        )
                    nc.tensor.matmul(t, lhsT=ones1, rhs=bv_sb,
                                     start=False, stop=True)
                    psj = t.rearrange("p (j d) -> p j d", d=DV)
                    nc.vector.tensor_copy(
                        out=v1[:, mt, 0::2, 0:DV], in_=psj[:, 0::2, :])
                    nc.vector.tensor_copy(
                        out=v1[:, mt, 1::2, 64:64 + DV], in_=psj[:, 1::2, :])

            with nc.named_scope("proj_v_seed"):
                for mt in range(4):
                    pv_unit(mt, tag=TAGS[mt % 4])

            # ---------- filler units (run inside attention windows) ----------
            def qk_unit(ft, nb, tag="F"):
                """Project one 512-wide n-block of q or k for head pair 1."""
                with nc.named_scope(f"fqk_{ft}_{nb}"):
                    t = ps_p.tile([P, NB], f32, tag=tag)
                    for kt in range(KT):
                        nc.tensor.matmul(
                            t,
                            lhsT=wqk_sb[:, kt, ft * P:(ft + 1) * P],
                            rhs=xT_sb[:, kt, nb * NB:(nb + 1) * NB],
                            start=(kt == 0), stop=(kt == KT - 1),
                        )
                    nc.vector.tensor_scalar_add(
                        out=qkT[:, ft, nb * NB:(nb + 1) * NB],
                        in0=t, scalar1=bqk_sb[:, ft:ft + 1])

            _op_i = [0]

            def op_unit(p, nt, tag="F", act_copy=None):
                """Output-project one 128-row n-block of head pair p."""
                with nc.named_scope(f"fop_{p}_{nt}"):
                    t = ps_p.tile([P, CH], f32, tag=tag)
                    for ds in range(2):
                        nc.tensor.matmul(
                            t[:, ds * NB:(ds + 1) * NB],
                            lhsT=valT[p][:, nt * P:(nt + 1) * P],
                            rhs=wout_sb[:, p, ds * NB:(ds + 1) * NB],
                            start=True, stop=True,
                        )
                    ob = stage_p.tile([P, CH], bf16)
                    # ACT copies only when ACT is idle (tail): mid-window they
                    # queue behind every remaining exp and stall the stores
                    if act_copy is None:
                        act_copy = False
                    if act_copy:
                        nc.scalar.copy(out=ob, in_=t)
                    else:
                        nc.vector.tensor_copy(out=ob, in_=t)
                    _op_i[0] += 1
                    nc.sync.dma_start(out=outp[p][nt * P:(nt + 1) * P, :], in_=ob)

            # ---------- attention window: one (pair, chunk, head) ----------
            # Windows alternate their PV accumulator between PSUM tags V and F
            # (double-buffered); fillers use the off-duty tag. Each window's
            # normalization is emitted inside the NEXT window (as a pre-PV
            # filler at iteration 2) so the window boundary never injects
            # DVE latency into the exp->mul->PV chain.
            _widx = [0]

            def attn(p, c, oe, fillers=(), pre=None, last=False, off=0, W=CH):
                j = 2 * p + oe
                base = oe * 64
                vtag = "VF"[_widx[0] % 2]
                ftag = "VF"[1 - _widx[0] % 2]
                _widx[0] += 1
                fill = {}
                if fillers:
                    step = max(1, 10 // len(fillers))
                    for i, f in enumerate(fillers):
                        fill.setdefault(min(13, 4 + i * step), []).append(
                            lambda f=f: f(ftag))
                pre = pre or {}
                col = c * CH + off
                with nc.named_scope(f"attn_{p}_{c}_{oe}_{off}"):
                    psv = ps_p.tile([P, W], f32, tag=vtag,
                                    name=f"psv_{p}_{c}_{oe}_{off}")
                    for mt in range(MT):
                        psl = ps_p.tile([P, W], f32, tag="AB"[mt % 2])
                        for h2 in range(W // NB):
                            nc.tensor.matmul(
                                psl[:, h2 * NB:(h2 + 1) * NB],
                                lhsT=qkT[base:base + 64, 2 + p,
                                         mt * P:(mt + 1) * P],
                                rhs=qkT[base:base + 64, p,
                                        col + h2 * NB:col + (h2 + 1) * NB],
                                start=True, stop=True,
                            )
                        wt = wT_p.tile([P, W], bf16)
                        nc.scalar.activation(out=wt, in_=psl, func=AF.Exp)
                        nc.vector.tensor_mul(out=wt, in0=wt,
                                             in1=em_sb[:, c, mt, off:off + W])
                        for f in pre.get(mt, ()):
                            f(ftag)
                        for h2 in range(W // NB):
                            nc.tensor.matmul(
                                psv[:, h2 * NB:(h2 + 1) * NB],
                                lhsT=v1[:, mt, j, :],
                                rhs=wt[:, h2 * NB:(h2 + 1) * NB],
                                start=(mt == 0), stop=(mt == MT - 1),
                            )
                        for f in fill.get(mt, ()):
                            f()

                def finish(_tag=None):
                    # normalization: recip the replicated Z rows off psv,
                    # DMA-move across partitions (DVE is lane-locked), copy
                    # the value half out of PSUM, multiply on Pool.
                    with nc.named_scope(f"norm_{p}_{c}_{oe}_{off}"):
                        vb, zb = (0, 64) if oe == 0 else (64, 0)
                        cs = slice(col, col + W)
                        zr = z_p.tile([P, W], f32, tag="zr")
                        nc.vector.reciprocal(
                            out=zr[zb:zb + 64, :], in_=psv[zb:zb + 64, :])
                        if last:
                            # final window: move the recip block across
                            # partitions with an identity matmul (PE and PSUM
                            # are free here — ~0.2us vs ~3.3us for the DMA
                            # move) and multiply straight out of PSUM.
                            zmp = ps_p.tile([P, W], f32, tag=ftag, name="zmp")
                            nc.tensor.matmul(
                                zmp[vb:vb + 64, :],
                                lhsT=ident_sb[zb:zb + 64, :],
                                rhs=zr[zb:zb + 64, :],
                                start=True, stop=True,
                            )
                            # only one PSUM operand allowed per DVE op: the
                            # value half goes through SBUF
                            vc = z_p.tile([P, W], f32, tag="pc")
                            nc.vector.tensor_copy(
                                out=vc[vb:vb + 64, :], in_=psv[vb:vb + 64, :])
                            h = W // 2
                            nc.vector.tensor_mul(
                                out=valT[p][vb:vb + 64, cs][:, 0:h],
                                in0=vc[vb:vb + 64, 0:h],
                                in1=zmp[vb:vb + 64, 0:h])
                            nc.vector.tensor_mul(
                                out=valT[p][vb:vb + 64, cs][:, h:W],
                                in0=vc[vb:vb + 64, h:W],
                                in1=zmp[vb:vb + 64, h:W])
                        else:
                            zm = z_p.tile([P, W], f32, tag="zm")
                            # issue via SWDGE (gpsimd): never queues behind
                            # the output stores on the sync queue
                            nc.gpsimd.dma_start(
                                out=zm[vb:vb + 64, :], in_=zr[zb:zb + 64, :])
                            vc = z_p.tile([P, W], f32, tag="pc")
                            nc.vector.tensor_copy(
                                out=vc[vb:vb + 64, :], in_=psv[vb:vb + 64, :])
                            nc.gpsimd.tensor_mul(
                                out=valT[p][vb:vb + 64, cs],
                                in0=vc[vb:vb + 64, :], in1=zm[vb:vb + 64, :])

                if last:
                    finish()
                    return None
                return finish

            # ---------- window schedule with interleaved fillers ----------
            # window 0 carries the rest of the v projection as pre-PV fillers
            # (v1[mt] must exist before its own PV consumes it; 4-ahead lead)
            def qk_f(ft, nb):
                return lambda tag: qk_unit(ft, nb, tag)

            def op_f(p, nt):
                return lambda tag: op_unit(p, nt, tag)

            n0 = attn(0, 0, 0,
                      pre={mt: [lambda tag, mt=mt: pv_unit(mt + 4, tag)]
                           for mt in range(MT - 4)})
            n1 = attn(0, 0, 1, [qk_f(1, nb) for nb in range(4)],
                      pre={2: [n0]})
            n2 = attn(0, 1, 0, [qk_f(3, 0), qk_f(3, 1),
                                op_f(0, 0), op_f(0, 1)],
                      pre={2: [n1]})
            n3 = attn(0, 1, 1, [qk_f(3, 2), qk_f(3, 3),
                                op_f(0, 2), op_f(0, 3)],
                      pre={2: [n2]})
            n4 = attn(1, 0, 0, [op_f(0, nt) for nt in (4, 5, 6, 7, 8, 9)],
                      pre={2: [n3]})
            n5 = attn(1, 0, 1, [op_f(0, nt) for nt in (10, 11, 12, 13, 14, 15)],
                      pre={2: [n4]})
            n6 = attn(1, 1, 0, [op_f(1, nt) for nt in range(6)],
                      pre={2: [n5]})
            # final window splits into two 512-wide halves: half A's norm and
            # output projection overlap half B's compute, shrinking the tail
            n7a = attn(1, 1, 1, [op_f(1, 6), op_f(1, 7)],
                       pre={2: [n6]}, off=0, W=NB)
            attn(1, 1, 1, [op_f(1, nt) for nt in (8, 9, 10, 11)],
                 pre={2: [n7a]}, last=True, off=NB, W=NB)
            # tail: final 512 columns of pair 1, rotating through freed PSUM
            # tags; copies alternate DVE/ACT (both idle by now)
            with nc.named_scope("op_tail"):
                for i, nt in enumerate(range(12, 16)):
                    op_unit(1, nt, tag=["A", "B", "V", "F"][i % 4],
                            act_copy=(i % 2 == 1))

    nc.compile()
    return nc


def get_nc():
    if "nc" not in _BUILT:
        _BUILT["nc"] = _build_nc()
    return _BUILT["nc"]


def _shard_inputs(stream, mask, w_qkv, b_qkv, w_out):
    """Build the 8 per-core input maps (host-side layout transforms)."""
    stream = np.asarray(stream, np.float32)
    mask = np.asarray(mask, np.float32)
    w_qkv = np.asarray(w_qkv, np.float32)
    b_qkv = np.asarray(b_qkv, np.float32)
    w_out = np.asarray(w_out, np.float32)

    emT = np.exp(mask[0].T).astype(BF16)  # (N, N) exp of transposed mask
    xT = [np.ascontiguousarray(stream[:, b, :].T).astype(BF16) for b in range(B)]

    in_maps = []
    for d in range(N_CORES):
        b = d // 4
        heads = [(d % 4) * 4 + j for j in range(NH)]
        qc = [w_qkv[:, h * DHEAD:h * DHEAD + DKQ] for h in heads]
        kc = [w_qkv[:, h * DHEAD + DKQ:h * DHEAD + 2 * DKQ] for h in heads]
        vc = [w_qkv[:, h * DHEAD + 2 * DKQ:(h + 1) * DHEAD] for h in heads]
        wqk = np.ascontiguousarray(np.concatenate(
            [qc[0], qc[1], qc[2], qc[3], kc[0], kc[1], kc[2], kc[3]],
            axis=1)).astype(BF16)
        wv = np.ascontiguousarray(np.concatenate(vc, axis=1)).astype(BF16)
        bq = [b_qkv[h * DHEAD:h * DHEAD + DKQ] for h in heads]
        bk = [b_qkv[h * DHEAD + DKQ:h * DHEAD + 2 * DKQ] for h in heads]
        bvv = [b_qkv[h * DHEAD + 2 * DKQ:(h + 1) * DHEAD] for h in heads]
        bqk_arr = np.stack(
            [np.concatenate([bq[0], bq[1]]), np.concatenate([bq[2], bq[3]]),
             np.concatenate([bk[0], bk[1]]), np.concatenate([bk[2], bk[3]])],
            axis=1).astype(np.float32)
        bv_arr = np.ascontiguousarray(np.concatenate(bvv)[None, :]).astype(BF16)
        woutd = np.ascontiguousarray(np.concatenate(
            [w_out[h * DV:(h + 1) * DV, :] for h in heads], axis=0)).astype(BF16)
        in_maps.append({
            "xT": xT[b], "wqk": wqk, "wv": wv, "bqk": bqk_arr, "bv": bv_arr,
            "ones": np.ones((1, P), BF16), "emT": emT, "wout": woutd,
            "ident": np.vstack([np.eye(64), np.eye(64)]).astype(np.float32),
        })
    return in_maps


def kernel(stream, mask, w_qkv, b_qkv, w_out, b_out):
    nc = get_nc()
    in_maps = _shard_inputs(stream, mask, w_qkv, b_qkv, w_out)
    res = run_bass_kernel_spmd(nc, in_maps, core_ids=list(range(N_CORES)))
    b_out = np.asarray(b_out, np.float32)
    out = np.empty((N, B, DSTR), np.float32)
    for b in range(B):
        acc = res.results[4 * b]["out0"].astype(np.float32)
        acc += res.results[4 * b]["out1"].astype(np.float32)
        for i in range(1, 4):
            acc += res.results[4 * b + i]["out0"].astype(np.float32)
            acc += res.results[4 * b + i]["out1"].astype(np.float32)
        out[:, b, :] = acc + b_out
    return out
